# revision 17
# baseline (speedup 1.0000x reference)
"""Trainium2 Bass kernel for nn_CascadedAttention_76836964925817.

Math: the reference module's attention machinery is dead code — softmax over a
size-1 axis is identically 1, so `context = x[0].sum(axis=0)` is a constant
and the layer reduces to the 28-dim nonlinear recurrence

    y[t] = sigmoid(Wo @ y[t-1] + Uo @ x[t-1] + c),   c = Co @ sum_t x[t],
    y[-1] = 0, x[-1] := 0.

The map y -> sigmoid(Wo y + b) is a strong contraction (measured Jacobian
2-norm <= 0.055), so each core solves its own 256-timestep slice from a cold
start with a W=4 column warmup — no cross-core state is needed.

Collective-free design (a collective's rendezvous wait absorbs inter-core
launch skew into the first core's measured exec time): every core receives
the FULL x (8.4 MB fp32, column-permuted on the host so one SPMD program
works for all cores) and computes the global sum itself:

  * per-core input xall (128, 8, 2054): d-major chunks; cols [0,260) are the
    core's local window x[t0-4 .. t0+254] (fed to the U matmuls), cols
    [260,2054) are all remaining timesteps in arbitrary order.
  * The global sum_t x[t] is one free-axis sum per chunk over cols [3,2054),
    split across VectorE (tensor_reduce) and ScalarE (activation accum_out)
    so every chunk's sum hides behind the per-chunk DMA stream.  The last
    chunk is DMA'd and summed in two pieces so the final c-dependency is a
    small 514-column tail.
  * U window: 8 fp32 matmuls (Uo.T chunks vs window cols) accumulated in
    PSUM; one extra identity matmul adds E (E = -500 on warmup cols for
    core 0 only, making its pre-t=0 state decay to the true zero init).
  * c: tiny fp32 matmuls Co.T chunks vs the per-chunk sums.
  * recurrence: all fp32 (f32r/bf16 matmuls round operands to ~bf16
    mantissa — measured max_rel 1e-2 through the U row — so the 4x slower
    fp32 matmul is the right trade at this size).  The moving stack is
    [Y; U] (64 rows) with c riding each sigmoid ACT's per-partition bias.
    A sigmoid warm-init ACT seeds Y; then S=2 Jacobi sweeps, each one fp32
    matmul with the constant stationary [Wo.T;0;I;0] plus one sigmoid ACT.
    The final sigmoid writes the output tile covering y[t0 .. t0+255].

All constants ride in a single packed (128, 508) tensor -> one DMA.
"""

import numpy as np

import concourse.bass as bass
import concourse.mybir as mybir
import concourse.tile as tile
from concourse import bacc
from concourse import bass_utils

F32 = mybir.dt.float32
F32R = mybir.dt.float32r
AF = mybir.ActivationFunctionType
ALU = mybir.AluOpType

T, D, V = 2048, 1024, 28
N_CORES = 8
TC = T // N_CORES        # 256 output timesteps per core
W = 4                    # warmup columns
NW = TC + W              # 260 window columns (U matmul width, even)
XCW = 2054               # per-chunk input cols: 260 window + 1794 complement
XSPL = 1796              # last chunk's DMA/sum split point (258-col tail)
DCH = D // 128           # 8 contraction chunks
S_SWEEPS = 2             # Jacobi sweeps after the sigmoid warm-init
E_NEG = -500.0           # warmup bias (must be < -(max|c| + margin) ~ -170)

# packed consts layout (128, 508)
C_UOT = 0                # [0, 224): Uo.T chunks
C_COT = DCH * V          # [224, 448): Co.T chunks
C_WOIS = 2 * DCH * V     # [448, 476): [Wo.T;0; I;0] rows 0-63
C_IDENT = C_WOIS + V     # [476, 504): I28 rows 0-27
C_E = C_IDENT + V        # [504, 508): E rows 0-27
C_TOT = C_E + W


def build_body(nc, xall, consts, yg, tc=None):
    t = tc
    from contextlib import ExitStack
    ctx = ExitStack()
    sbp = ctx.enter_context(t.tile_pool(name="sb", bufs=1))
    pp = ctx.enter_context(t.tile_pool(name="pp", bufs=1, space="PSUM"))

    def st(shape, name, dt=F32):
        return sbp.tile(shape, dt, name=name, tag=name)

    xall_sb = st([128, DCH, XCW], "xall_sb")
    consts_sb = st([128, C_TOT], "consts_sb")
    sred = st([128, DCH + 1], "sred")
    cbias = st([V, 1], "cbias")
    m_sb = st([64, NW + 2], "m_sb")
    yout = st([V, TC], "yout")
    scr_s = st([128, XCW - 3], "scr_s")
    dummy = st([1, 1], "dummy")

    psU = pp.tile([V, NW], F32, name="psU", tag="psU")
    psC = pp.tile([V, 2], F32, name="psC", tag="psC")
    psZ = pp.tile([V, NW], F32, name="psZ", tag="psZ")

    uot = lambda c: consts_sb[:, C_UOT + c * V:C_UOT + (c + 1) * V]
    cot = lambda c: consts_sb[:, C_COT + c * V:C_COT + (c + 1) * V]
    wois = consts_sb[0:64, C_WOIS:C_WOIS + V]
    ident = consts_sb[0:V, C_IDENT:C_IDENT + V]
    esrc = consts_sb[0:V, C_E:C_E + W]

    # Early dummy sigmoid so the ACT table load happens off the critical path.
    nc.vector.memset(dummy[:, :], 0.0)
    nc.scalar.activation(out=dummy[:, :], in_=dummy[:, :], func=AF.Sigmoid)
    # Y region must start as zeros (cold-start warmup state).
    nc.vector.memset(m_sb[:, :], 0.0)

    # ---------------- DMAs: packed consts, then the x chunk stream ---------
    nc.sync.dma_start(consts_sb[:, :], consts)
    xv = xall.rearrange("p (c j) -> p c j", c=DCH)
    for c in range(DCH - 1):
        nc.sync.dma_start(xall_sb[:, c, :], xv[:, c, :])
    nc.sync.dma_start(xall_sb[:, DCH - 1, 0:XSPL], xv[:, DCH - 1, 0:XSPL])
    nc.sync.dma_start(xall_sb[:, DCH - 1, XSPL:XCW], xv[:, DCH - 1, XSPL:XCW])

    # ---------------- U = Uo @ window, + E on the warmup cols --------------
    for c in range(DCH):
        nc.tensor.matmul(
            psU[:, :],
            lhsT=uot(c),
            rhs=xall_sb[:, c, 0:NW],
            start=(c == 0),
            stop=False,
        )
    nc.tensor.matmul(
        psU[:, 0:W], lhsT=ident, rhs=esrc, start=False, stop=True,
    )

    # ---------------- global sum (DVE + ScalarE) + c -----------------------
    # Each chunk's sum covers window cols [3,260) (every t in [t0-1, t0+254]
    # exactly once; cols 0-2 are U-halo duplicates) plus the complement —
    # together every timestep exactly once.  Chunks alternate engines; the
    # split last chunk puts only a 514-col tail on the critical path.
    def emit_sum(eng, src, dst):
        if eng == 'v':
            nc.vector.tensor_reduce(out=dst, in_=src,
                                    axis=mybir.AxisListType.X, op=ALU.add)
        else:
            nc.scalar.activation(out=scr_s[:, 0:src.shape[-1]], in_=src,
                                 func=AF.Copy, accum_out=dst)

    for c in range(DCH - 1):
        emit_sum('v' if c % 2 == 0 else 's',
                 xall_sb[:, c, 3:XCW], sred[:, c:c + 1])
    emit_sum('v', xall_sb[:, DCH - 1, 3:XSPL], sred[:, DCH - 1:DCH])
    emit_sum('s', xall_sb[:, DCH - 1, XSPL:XCW], sred[:, DCH:DCH + 1])

    for c in range(DCH):
        last = c == DCH - 1
        nc.tensor.matmul(
            psC[:, 0:2] if last else psC[:, 0:1],
            lhsT=cot(c),
            rhs=sred[:, c:c + 2] if last else sred[:, c:c + 1],
            start=(c == 0),
            stop=last,
        )
    nc.vector.tensor_reduce(out=cbias[:, :], in_=psC[:, 0:2],
                            axis=mybir.AxisListType.X, op=ALU.add)

    # -------- stage U (DVE, parallel to ScalarE warm-init) -----------------
    nc.vector.tensor_copy(m_sb[32:32 + V, 0:NW], psU[:, :])
    nc.scalar.activation(out=m_sb[0:V, 1:NW + 1], in_=psU[:, :],
                         func=AF.Sigmoid, bias=cbias[:, 0:1], scale=1.0)

    # ---------------- Jacobi sweeps ---------------------------------------
    for s in range(S_SWEEPS):
        nc.tensor.matmul(
            psZ[:, :],
            lhsT=wois,
            rhs=m_sb[0:64, 0:NW],
            start=True,
            stop=True,
        )
        if s < S_SWEEPS - 1:
            nc.scalar.activation(out=m_sb[0:V, 1:NW + 1], in_=psZ[:, :],
                                 func=AF.Sigmoid, bias=cbias[:, 0:1],
                                 scale=1.0)
        else:
            # final sweep writes full-f32 output (no f32r rounding of y)
            nc.scalar.activation(out=yout[:, :], in_=psZ[:, W - 1:NW - 1],
                                 func=AF.Sigmoid, bias=cbias[:, 0:1],
                                 scale=1.0)

    # ---------------- write output ----------------------------------------
    nc.sync.dma_start(yg, yout[:, :])
    ctx.close()


_CACHED_NC = {}


def _get_nc():
    if "nc" not in _CACHED_NC:
        nc = bacc.Bacc("TRN2", target_bir_lowering=False, debug=False,
                       num_devices=N_CORES)
        xall = nc.dram_tensor("xall", [128, DCH * XCW], F32,
                              kind="ExternalInput")
        consts = nc.dram_tensor("consts", [128, C_TOT], F32,
                                kind="ExternalInput")
        yg = nc.dram_tensor("yg", [V, TC], F32, kind="ExternalOutput")
        with tile.TileContext(nc) as t:
            build_body(nc, xall.ap(), consts.ap(), yg.ap(), tc=t)
        nc.compile()
        _CACHED_NC["nc"] = nc
    return _CACHED_NC["nc"]


def _to_dev_layout(buf):
    """(cols, D) -> (128, DCH*cols): dev[p, c*cols+j] = buf[j, 128c+p]."""
    cols = buf.shape[0]
    return np.ascontiguousarray(
        buf.T.reshape(DCH, 128, cols).transpose(1, 0, 2).reshape(128, -1))


def make_in_maps(x, Uo, Co, Wo):
    xb = np.ascontiguousarray(np.asarray(x, np.float32)[0])        # (T, D)
    Uo = np.asarray(Uo, np.float32)
    Co = np.asarray(Co, np.float32)
    Wo = np.asarray(Wo, np.float32)

    cbase = np.zeros((128, C_TOT), np.float32)
    cbase[:, C_UOT:C_UOT + DCH * V] = _to_dev_layout(Uo)
    cbase[:, C_COT:C_COT + DCH * V] = _to_dev_layout(Co)
    cbase[0:V, C_WOIS:C_WOIS + V] = Wo.T
    cbase[32:32 + V, C_WOIS:C_WOIS + V] = np.eye(V, dtype=np.float32)
    cbase[0:V, C_IDENT:C_IDENT + V] = np.eye(V, dtype=np.float32)

    in_maps = []
    for r in range(N_CORES):
        t0 = r * TC
        buf = np.zeros((XCW, D), np.float32)
        # window cols w=0..258 <-> x[t0-4+w]; col 259 stays zero
        lo = t0 - W
        src_lo = max(0, lo)
        buf[src_lo - lo:NW - 1] = xb[src_lo:t0 + TC - 1]
        # complement: every t outside [t0-1, t0+254]
        comp = np.concatenate([np.arange(0, max(0, t0 - 1)),
                               np.arange(t0 + TC - 1, T)])
        buf[NW:NW + len(comp)] = xb[comp]
        consts = cbase.copy()
        if r == 0:
            consts[0:V, C_E:C_E + W - 1] = E_NEG
        in_maps.append({"xall": _to_dev_layout(buf), "consts": consts})
    return in_maps


def unshard_output(results):
    y = np.empty((T, V), np.float32)
    for r in range(N_CORES):
        y[r * TC:(r + 1) * TC, :] = results[r]["yg"].T
    return y[None]


def run(inputs, trace=False, **kw):
    nc = _get_nc()
    in_maps = make_in_maps(inputs["x"], inputs["Uo"], inputs["Co"],
                           inputs["Wo"])
    res = bass_utils.run_bass_kernel_spmd(
        nc, in_maps, core_ids=list(range(N_CORES)), trace=trace, **kw)
    return unshard_output(res.results), res


def kernel(**inputs):
    out, _ = run(inputs)
    return out


# revision 18
# speedup vs baseline: 1.8306x; 1.8306x over previous
"""Trainium2 Bass kernel for nn_CascadedAttention_76836964925817.

Math: the reference module's attention machinery is dead code — softmax over a
size-1 axis is identically 1, so `context = x[0].sum(axis=0)` is a constant
and the layer reduces to the 28-dim nonlinear recurrence

    y[t] = sigmoid(Wo @ y[t-1] + Uo @ x[t-1] + c),   c = Co @ sum_t x[t],
    y[-1] = 0, x[-1] := 0.

The map y -> sigmoid(Wo y + b) is a strong contraction (measured Jacobian
2-norm <= 0.055), so each core solves its own 256-timestep slice from a cold
start with a W=4 column warmup — no cross-core state is needed.

Collective-free design (a collective's rendezvous wait absorbs inter-core
launch skew into the first core's measured exec time): every core receives
the FULL x (8.4 MB fp32, column-permuted on the host so one SPMD program
works for all cores) and computes the global sum itself:

  * per-core input xall (128, 8, 2054): d-major chunks; cols [0,260) are the
    core's local window x[t0-4 .. t0+254] (fed to the U matmuls), cols
    [260,2054) are all remaining timesteps in arbitrary order.
  * The global sum_t x[t] is one free-axis sum per chunk over cols [3,2054),
    split across VectorE (tensor_reduce) and ScalarE (activation accum_out)
    so every chunk's sum hides behind the per-chunk DMA stream.  The last
    chunk is DMA'd and summed in two pieces so the final c-dependency is a
    small 514-column tail.
  * U window: 8 fp32 matmuls (Uo.T chunks vs window cols) accumulated in
    PSUM; one extra identity matmul adds E (E = -500 on warmup cols for
    core 0 only, making its pre-t=0 state decay to the true zero init).
  * c: tiny fp32 matmuls Co.T chunks vs the per-chunk sums.
  * recurrence: the sweep matmul runs in bf16 (1 cyc/col vs fp32's ~10),
    made accurate by structure: U is carried as bf16 hi + exact residual lo
    rows (so the U term is f32-grade), c rides each sigmoid ACT's
    per-partition f32 bias, and only the contracting Y term sees bf16
    rounding (delta_z ~ 1e-3, 20x under the gate).  Moving stack
    [Y; U_hi; U_lo] (96 rows), stationary [Wo.T;0; I;0; I;0] bf16.
    A sigmoid warm-init ACT seeds Y; S=2 Jacobi sweeps (one matmul + one
    sigmoid ACT each); the final sigmoid writes a full-f32 output tile.

All constants ride in a single packed (128, 508) tensor -> one DMA.
"""

import numpy as np

import concourse.bass as bass
import concourse.mybir as mybir
import concourse.tile as tile
from concourse import bacc
from concourse import bass_utils

F32 = mybir.dt.float32
F32R = mybir.dt.float32r
AF = mybir.ActivationFunctionType
ALU = mybir.AluOpType

T, D, V = 2048, 1024, 28
N_CORES = 8
TC = T // N_CORES        # 256 output timesteps per core
W = 4                    # warmup columns
NW = TC + W              # 260 window columns (U matmul width, even)
XCW = 2054               # per-chunk input cols: 260 window + 1794 complement
XSPL1 = 1413             # last chunk's 2nd DMA/sum split point
XSPL2 = 2054             # (last piece is [XSPL1, XCW))
DCH = D // 128           # 8 contraction chunks
S_SWEEPS = 2             # Jacobi sweeps after the sigmoid warm-init
E_NEG = -500.0           # warmup bias (must be < -(max|c| + margin) ~ -170)

# packed consts layout (128, 508)
C_UOT = 0                # [0, 224): Uo.T chunks
C_COT = DCH * V          # [224, 448): Co.T chunks
C_WOIS = 2 * DCH * V     # [448, 476): unused (wois16 is a separate input)
C_IDENT = C_WOIS + V     # [476, 504): I28 rows 0-27
C_E = C_IDENT + V        # [504, 508): E rows 0-27
C_TOT = C_E + W


def build_body(nc, xall, consts, wois16, yg, tc=None):
    t = tc
    from contextlib import ExitStack
    ctx = ExitStack()
    sbp = ctx.enter_context(t.tile_pool(name="sb", bufs=1))
    pp = ctx.enter_context(t.tile_pool(name="pp", bufs=1, space="PSUM"))

    def st(shape, name, dt=F32):
        return sbp.tile(shape, dt, name=name, tag=name)

    xall_sb = st([128, DCH, XCW], "xall_sb")
    consts_sb = st([128, C_TOT], "consts_sb")
    sred = st([128, DCH + 2], "sred")
    cbias = st([V, 1], "cbias")
    m_sb = st([96, NW + 2], "m_sb", mybir.dt.bfloat16)
    wois16_sb = st([96, V], "wois16_sb", mybir.dt.bfloat16)
    yout = st([V, TC], "yout")
    scr_s = st([128, XCW - 3], "scr_s")
    dummy = st([1, 1], "dummy")

    psU = pp.tile([V, NW], F32, name="psU", tag="psU")
    psC = pp.tile([V, 3], F32, name="psC", tag="psC")
    psZ = pp.tile([V, NW], F32, name="psZ", tag="psZ")

    uot = lambda c: consts_sb[:, C_UOT + c * V:C_UOT + (c + 1) * V]
    cot = lambda c: consts_sb[:, C_COT + c * V:C_COT + (c + 1) * V]
    ident = consts_sb[0:V, C_IDENT:C_IDENT + V]
    esrc = consts_sb[0:V, C_E:C_E + W]

    # Early dummy sigmoid so the ACT table load happens off the critical path.
    nc.vector.memset(dummy[:, :], 0.0)
    nc.scalar.activation(out=dummy[:, :], in_=dummy[:, :], func=AF.Sigmoid)
    # Y region must start as zeros (cold-start warmup state).
    nc.vector.memset(m_sb[:, :], 0.0)

    # ---------------- DMAs: packed consts, then the x chunk stream ---------
    nc.sync.dma_start(consts_sb[:, :], consts)
    nc.sync.dma_start(wois16_sb[:, :], wois16)
    xv = xall.rearrange("p (c j) -> p c j", c=DCH)
    L = DCH - 1
    nc.sync.dma_start(xall_sb[:, L, 0:NW], xv[:, L, 0:NW])
    for c in range(DCH - 1):
        nc.sync.dma_start(xall_sb[:, c, :], xv[:, c, :])
    nc.sync.dma_start(xall_sb[:, L, NW:XSPL1], xv[:, L, NW:XSPL1])
    nc.sync.dma_start(xall_sb[:, L, XSPL1:XCW], xv[:, L, XSPL1:XCW])

    # ---------------- U = Uo @ window, + E on the warmup cols --------------
    for c in range(DCH):
        nc.tensor.matmul(
            psU[:, :],
            lhsT=uot(c),
            rhs=xall_sb[:, c, 0:NW],
            start=(c == 0),
            stop=False,
        )
    nc.tensor.matmul(
        psU[:, 0:W], lhsT=ident, rhs=esrc, start=False, stop=True,
    )

    # ---------------- global sum (DVE + ScalarE) + c -----------------------
    # Each chunk's sum covers window cols [3,260) (every t in [t0-1, t0+254]
    # exactly once; cols 0-2 are U-halo duplicates) plus the complement —
    # together every timestep exactly once.  Chunks alternate engines; the
    # split last chunk puts only a 514-col tail on the critical path.
    def emit_sum(eng, src, dst):
        if eng == 'v':
            nc.vector.tensor_reduce(out=dst, in_=src,
                                    axis=mybir.AxisListType.X, op=ALU.add)
        else:
            nc.scalar.activation(out=scr_s[:, 0:src.shape[-1]], in_=src,
                                 func=AF.Copy, accum_out=dst)

    emit_sum('v', xall_sb[:, L, 3:NW], sred[:, DCH - 1:DCH])
    for c in range(DCH - 1):
        emit_sum('v' if c % 2 == 0 else 's',
                 xall_sb[:, c, 3:XCW], sred[:, c:c + 1])
    emit_sum('s', xall_sb[:, L, NW:XSPL1], sred[:, DCH:DCH + 1])
    emit_sum('v', xall_sb[:, L, XSPL1:XCW], sred[:, DCH + 1:DCH + 2])

    for c in range(DCH):
        last = c == DCH - 1
        nc.tensor.matmul(
            psC[:, 0:3] if last else psC[:, 0:1],
            lhsT=cot(c),
            rhs=sred[:, c:c + 3] if last else sred[:, c:c + 1],
            start=(c == 0),
            stop=last,
        )
    nc.vector.tensor_reduce(out=cbias[:, :], in_=psC[:, 0:3],
                            axis=mybir.AxisListType.X, op=ALU.add)

    # -------- stage U as bf16 hi + exact residual lo (DVE) -----------------
    nc.vector.tensor_copy(m_sb[32:32 + V, 0:NW], psU[:, :])
    nc.vector.tensor_tensor(m_sb[64:64 + V, 0:NW], psU[:, :],
                            m_sb[32:32 + V, 0:NW], ALU.subtract)
    nc.scalar.activation(out=m_sb[0:V, 1:NW + 1], in_=psU[:, :],
                         func=AF.Sigmoid, bias=cbias[:, 0:1], scale=1.0)

    # ---------------- Jacobi sweeps ---------------------------------------
    for s in range(S_SWEEPS):
        nc.tensor.matmul(
            psZ[:, :],
            lhsT=wois16_sb[:, :],
            rhs=m_sb[0:96, 0:NW],
            start=True,
            stop=True,
        )
        if s < S_SWEEPS - 1:
            nc.scalar.activation(out=m_sb[0:V, 1:NW + 1], in_=psZ[:, :],
                                 func=AF.Sigmoid, bias=cbias[:, 0:1],
                                 scale=1.0)
        else:
            # final sweep writes full-f32 output (no f32r rounding of y)
            nc.scalar.activation(out=yout[:, :], in_=psZ[:, W - 1:NW - 1],
                                 func=AF.Sigmoid, bias=cbias[:, 0:1],
                                 scale=1.0)

    # ---------------- write output ----------------------------------------
    nc.sync.dma_start(yg, yout[:, :])
    ctx.close()


_CACHED_NC = {}


def _get_nc():
    if "nc" not in _CACHED_NC:
        nc = bacc.Bacc("TRN2", target_bir_lowering=False, debug=False,
                       num_devices=N_CORES)
        xall = nc.dram_tensor("xall", [128, DCH * XCW], F32,
                              kind="ExternalInput")
        consts = nc.dram_tensor("consts", [128, C_TOT], F32,
                                kind="ExternalInput")
        wois16 = nc.dram_tensor("wois16", [96, V], mybir.dt.bfloat16,
                                kind="ExternalInput")
        yg = nc.dram_tensor("yg", [V, TC], F32, kind="ExternalOutput")
        with tile.TileContext(nc) as t:
            build_body(nc, xall.ap(), consts.ap(), wois16.ap(), yg.ap(), tc=t)
        nc.compile()
        _CACHED_NC["nc"] = nc
    return _CACHED_NC["nc"]


def _to_dev_layout(buf):
    """(cols, D) -> (128, DCH*cols): dev[p, c*cols+j] = buf[j, 128c+p]."""
    cols = buf.shape[0]
    return np.ascontiguousarray(
        buf.T.reshape(DCH, 128, cols).transpose(1, 0, 2).reshape(128, -1))


def make_in_maps(x, Uo, Co, Wo):
    xb = np.ascontiguousarray(np.asarray(x, np.float32)[0])        # (T, D)
    Uo = np.asarray(Uo, np.float32)
    Co = np.asarray(Co, np.float32)
    Wo = np.asarray(Wo, np.float32)

    cbase = np.zeros((128, C_TOT), np.float32)
    cbase[:, C_UOT:C_UOT + DCH * V] = _to_dev_layout(Uo)
    cbase[:, C_COT:C_COT + DCH * V] = _to_dev_layout(Co)
    cbase[0:V, C_IDENT:C_IDENT + V] = np.eye(V, dtype=np.float32)
    import ml_dtypes
    wois16 = np.zeros((96, V), ml_dtypes.bfloat16)
    wois16[0:V] = Wo.T.astype(ml_dtypes.bfloat16)
    wois16[32:32 + V] = np.eye(V, dtype=ml_dtypes.bfloat16)
    wois16[64:64 + V] = np.eye(V, dtype=ml_dtypes.bfloat16)

    in_maps = []
    for r in range(N_CORES):
        t0 = r * TC
        buf = np.zeros((XCW, D), np.float32)
        # window cols w=0..258 <-> x[t0-4+w]; col 259 stays zero
        lo = t0 - W
        src_lo = max(0, lo)
        buf[src_lo - lo:NW - 1] = xb[src_lo:t0 + TC - 1]
        # complement: every t outside [t0-1, t0+254]
        comp = np.concatenate([np.arange(0, max(0, t0 - 1)),
                               np.arange(t0 + TC - 1, T)])
        buf[NW:NW + len(comp)] = xb[comp]
        consts = cbase.copy()
        if r == 0:
            consts[0:V, C_E:C_E + W - 1] = E_NEG
        in_maps.append({"xall": _to_dev_layout(buf), "consts": consts,
                        "wois16": wois16})
    return in_maps


def unshard_output(results):
    y = np.empty((T, V), np.float32)
    for r in range(N_CORES):
        y[r * TC:(r + 1) * TC, :] = results[r]["yg"].T
    return y[None]


def run(inputs, trace=False, **kw):
    nc = _get_nc()
    in_maps = make_in_maps(inputs["x"], inputs["Uo"], inputs["Co"],
                           inputs["Wo"])
    res = bass_utils.run_bass_kernel_spmd(
        nc, in_maps, core_ids=list(range(N_CORES)), trace=trace, **kw)
    return unshard_output(res.results), res


def kernel(**inputs):
    out, _ = run(inputs)
    return out


# revision 19
# speedup vs baseline: 1.9403x; 1.0599x over previous
"""Trainium2 Bass kernel for nn_CascadedAttention_76836964925817.

Math: the reference module's attention machinery is dead code — softmax over a
size-1 axis is identically 1, so `context = x[0].sum(axis=0)` is a constant
and the layer reduces to the 28-dim nonlinear recurrence

    y[t] = sigmoid(Wo @ y[t-1] + Uo @ x[t-1] + c),   c = Co @ sum_t x[t],
    y[-1] = 0, x[-1] := 0.

The map y -> sigmoid(Wo y + b) is a strong contraction (measured Jacobian
2-norm <= 0.055), so each core solves its own 256-timestep slice from a cold
start with a W=4 column warmup — no cross-core state is needed.

Collective-free design (a collective's rendezvous wait absorbs inter-core
launch skew into the first core's measured exec time): every core receives
the FULL x (8.4 MB fp32, column-permuted on the host so one SPMD program
works for all cores) and computes the global sum itself:

  * per-core input xall (128, 8, 2054): d-major chunks; cols [0,260) are the
    core's local window x[t0-4 .. t0+254] (fed to the U matmuls), cols
    [260,2054) are all remaining timesteps in arbitrary order.
  * The global sum_t x[t] is one free-axis sum per chunk over cols [3,2054),
    split across VectorE (tensor_reduce) and ScalarE (activation accum_out)
    so every chunk's sum hides behind the per-chunk DMA stream.  The last
    chunk is DMA'd and summed in two pieces so the final c-dependency is a
    small 514-column tail.
  * U window: 8 fp32 matmuls (Uo.T chunks vs window cols) accumulated in
    PSUM; one extra identity matmul adds E (E = -500 on warmup cols for
    core 0 only, making its pre-t=0 state decay to the true zero init).
  * c: tiny fp32 matmuls Co.T chunks vs the per-chunk sums.
  * recurrence: the sweep matmul runs in bf16 (1 cyc/col vs fp32's ~10),
    made accurate by structure: U is carried as bf16 hi + exact residual lo
    rows (so the U term is f32-grade), c rides each sigmoid ACT's
    per-partition f32 bias, and only the contracting Y term sees bf16
    rounding (delta_z ~ 1e-3, 20x under the gate).  Moving stack
    [Y; U_hi; U_lo] (96 rows), stationary [Wo.T;0; I;0; I;0] bf16.
    A sigmoid warm-init ACT seeds Y; S=2 Jacobi sweeps (one matmul + one
    sigmoid ACT each); the final sigmoid writes a full-f32 output tile.

All constants ride in a single packed (128, 508) tensor -> one DMA.
"""

import numpy as np

import concourse.bass as bass
import concourse.mybir as mybir
import concourse.tile as tile
from concourse import bacc
from concourse import bass_utils

F32 = mybir.dt.float32
F32R = mybir.dt.float32r
AF = mybir.ActivationFunctionType
ALU = mybir.AluOpType

T, D, V = 2048, 1024, 28
N_CORES = 8
TC = T // N_CORES        # 256 output timesteps per core
W = 4                    # warmup columns
NW = TC + W              # 260 window columns (U matmul width, even)
XCW = 2054               # per-chunk input cols: 260 window + 1794 complement
XSPL1 = 1413             # last chunk's 2nd DMA/sum split point
XSPL2 = 2054             # (last piece is [XSPL1, XCW))
DCH = D // 128           # 8 contraction chunks
S_SWEEPS = 2             # Jacobi sweeps after the sigmoid warm-init
E_NEG = -500.0           # warmup bias (must be < -(max|c| + margin) ~ -170)

# packed consts layout (128, 508)
C_UOT = 0                # [0, 224): Uo.T chunks
C_COT = DCH * V          # [224, 448): Co.T chunks
C_WOIS = 2 * DCH * V     # [448, 476): unused (wois16 is a separate input)
C_IDENT = C_WOIS + V     # [476, 504): I28 rows 0-27
C_E = C_IDENT + V        # [504, 508): E rows 0-27
C_TOT = C_E + W


def build_body(nc, xall, consts, wois16, yg, tc=None):
    t = tc
    from contextlib import ExitStack
    ctx = ExitStack()
    sbp = ctx.enter_context(t.tile_pool(name="sb", bufs=1))
    pp = ctx.enter_context(t.tile_pool(name="pp", bufs=1, space="PSUM"))

    def st(shape, name, dt=F32):
        return sbp.tile(shape, dt, name=name, tag=name)

    xall_sb = st([128, DCH, XCW], "xall_sb")
    consts_sb = st([128, C_TOT], "consts_sb")
    sred = st([128, DCH + 2], "sred")
    cbias = st([V, 1], "cbias")
    m_sb = st([96, NW + 2], "m_sb", mybir.dt.bfloat16)
    wois16_sb = st([96, V], "wois16_sb", mybir.dt.bfloat16)
    yout = st([V, TC], "yout")
    scr_s = st([128, XCW - 3], "scr_s")
    dummy = st([1, 1], "dummy")

    psU = pp.tile([V, NW], F32, name="psU", tag="psU")
    psC = pp.tile([V, 3], F32, name="psC", tag="psC")
    psZ = pp.tile([V, NW], F32, name="psZ", tag="psZ")

    uot = lambda c: consts_sb[:, C_UOT + c * V:C_UOT + (c + 1) * V]
    cot = lambda c: consts_sb[:, C_COT + c * V:C_COT + (c + 1) * V]
    ident = consts_sb[0:V, C_IDENT:C_IDENT + V]
    esrc = consts_sb[0:V, C_E:C_E + W]

    # Early dummy sigmoid so the ACT table load happens off the critical path.
    nc.vector.memset(dummy[:, :], 0.0)
    nc.scalar.activation(out=dummy[:, :], in_=dummy[:, :], func=AF.Sigmoid)
    # Y region must start as zeros (cold-start warmup state).
    nc.vector.memset(m_sb[:, :], 0.0)

    # ---------------- DMAs: packed consts, then the x chunk stream ---------
    nc.sync.dma_start(consts_sb[:, :], consts)
    nc.sync.dma_start(wois16_sb[:, :], wois16)
    xv = xall.rearrange("p (c j) -> p c j", c=DCH)
    L = DCH - 1
    nc.sync.dma_start(xall_sb[:, L, 0:NW], xv[:, L, 0:NW])
    for c in range(DCH - 1):
        nc.sync.dma_start(xall_sb[:, c, :], xv[:, c, :])
    nc.sync.dma_start(xall_sb[:, L, NW:XSPL1], xv[:, L, NW:XSPL1])
    nc.sync.dma_start(xall_sb[:, L, XSPL1:XCW], xv[:, L, XSPL1:XCW])

    # ---------------- U = Uo @ window, + E on the warmup cols --------------
    for c in range(DCH):
        nc.tensor.matmul(
            psU[:, :],
            lhsT=uot(c),
            rhs=xall_sb[:, c, 0:NW],
            start=(c == 0),
            stop=False,
        )
    nc.tensor.matmul(
        psU[:, 0:W], lhsT=ident, rhs=esrc, start=False, stop=True,
    )

    # ---------------- global sum (DVE + ScalarE) + c -----------------------
    # Each chunk's sum covers window cols [3,260) (every t in [t0-1, t0+254]
    # exactly once; cols 0-2 are U-halo duplicates) plus the complement —
    # together every timestep exactly once.  Chunks alternate engines; the
    # split last chunk puts only a 514-col tail on the critical path.
    def emit_sum(eng, src, dst):
        if eng == 'v':
            nc.vector.tensor_reduce(out=dst, in_=src,
                                    axis=mybir.AxisListType.X, op=ALU.add)
        else:
            nc.scalar.activation(out=scr_s[:, 0:src.shape[-1]], in_=src,
                                 func=AF.Copy, accum_out=dst)

    emit_sum('v', xall_sb[:, L, 3:NW], sred[:, DCH - 1:DCH])
    for c in range(DCH - 1):
        emit_sum('s' if c in (2, 5) else 'v',
                 xall_sb[:, c, 3:XCW], sred[:, c:c + 1])
    emit_sum('s', xall_sb[:, L, NW:XSPL1], sred[:, DCH:DCH + 1])
    emit_sum('v', xall_sb[:, L, XSPL1:XCW], sred[:, DCH + 1:DCH + 2])

    for c in range(DCH):
        last = c == DCH - 1
        nc.tensor.matmul(
            psC[:, 0:3] if last else psC[:, 0:1],
            lhsT=cot(c),
            rhs=sred[:, c:c + 3] if last else sred[:, c:c + 1],
            start=(c == 0),
            stop=last,
        )
    nc.vector.tensor_reduce(out=cbias[:, :], in_=psC[:, 0:3],
                            axis=mybir.AxisListType.X, op=ALU.add)

    # -------- stage U as bf16 hi + exact residual lo (DVE) -----------------
    nc.vector.tensor_copy(m_sb[32:32 + V, 0:NW], psU[:, :])
    nc.vector.tensor_tensor(m_sb[64:64 + V, 0:NW], psU[:, :],
                            m_sb[32:32 + V, 0:NW], ALU.subtract)
    nc.scalar.activation(out=m_sb[0:V, 1:NW + 1], in_=psU[:, :],
                         func=AF.Sigmoid, bias=cbias[:, 0:1], scale=1.0)

    # ---------------- Jacobi sweeps ---------------------------------------
    for s in range(S_SWEEPS):
        nc.tensor.matmul(
            psZ[:, :],
            lhsT=wois16_sb[:, :],
            rhs=m_sb[0:96, 0:NW],
            start=True,
            stop=True,
        )
        if s < S_SWEEPS - 1:
            nc.scalar.activation(out=m_sb[0:V, 1:NW + 1], in_=psZ[:, :],
                                 func=AF.Sigmoid, bias=cbias[:, 0:1],
                                 scale=1.0)
        else:
            # final sweep writes the full-f32 output in two pieces so the
            # first piece's DMA receipt overlaps the second piece's ACT
            h = TC // 2
            nc.scalar.activation(out=yout[:, 0:h], in_=psZ[:, W - 1:W - 1 + h],
                                 func=AF.Sigmoid, bias=cbias[:, 0:1],
                                 scale=1.0)
            nc.sync.dma_start(yg[:, 0:h], yout[:, 0:h])
            nc.scalar.activation(out=yout[:, h:TC],
                                 in_=psZ[:, W - 1 + h:NW - 1],
                                 func=AF.Sigmoid, bias=cbias[:, 0:1],
                                 scale=1.0)

    # ---------------- write output ----------------------------------------
    nc.sync.dma_start(yg[:, TC // 2:TC], yout[:, TC // 2:TC])
    ctx.close()


_CACHED_NC = {}


def _get_nc():
    if "nc" not in _CACHED_NC:
        nc = bacc.Bacc("TRN2", target_bir_lowering=False, debug=False,
                       num_devices=N_CORES)
        xall = nc.dram_tensor("xall", [128, DCH * XCW], F32,
                              kind="ExternalInput")
        consts = nc.dram_tensor("consts", [128, C_TOT], F32,
                                kind="ExternalInput")
        wois16 = nc.dram_tensor("wois16", [96, V], mybir.dt.bfloat16,
                                kind="ExternalInput")
        yg = nc.dram_tensor("yg", [V, TC], F32, kind="ExternalOutput")
        with tile.TileContext(nc) as t:
            build_body(nc, xall.ap(), consts.ap(), wois16.ap(), yg.ap(), tc=t)
        nc.compile()
        _CACHED_NC["nc"] = nc
    return _CACHED_NC["nc"]


def _to_dev_layout(buf):
    """(cols, D) -> (128, DCH*cols): dev[p, c*cols+j] = buf[j, 128c+p]."""
    cols = buf.shape[0]
    return np.ascontiguousarray(
        buf.T.reshape(DCH, 128, cols).transpose(1, 0, 2).reshape(128, -1))


def make_in_maps(x, Uo, Co, Wo):
    xb = np.ascontiguousarray(np.asarray(x, np.float32)[0])        # (T, D)
    Uo = np.asarray(Uo, np.float32)
    Co = np.asarray(Co, np.float32)
    Wo = np.asarray(Wo, np.float32)

    cbase = np.zeros((128, C_TOT), np.float32)
    cbase[:, C_UOT:C_UOT + DCH * V] = _to_dev_layout(Uo)
    cbase[:, C_COT:C_COT + DCH * V] = _to_dev_layout(Co)
    cbase[0:V, C_IDENT:C_IDENT + V] = np.eye(V, dtype=np.float32)
    import ml_dtypes
    wois16 = np.zeros((96, V), ml_dtypes.bfloat16)
    wois16[0:V] = Wo.T.astype(ml_dtypes.bfloat16)
    wois16[32:32 + V] = np.eye(V, dtype=ml_dtypes.bfloat16)
    wois16[64:64 + V] = np.eye(V, dtype=ml_dtypes.bfloat16)

    in_maps = []
    for r in range(N_CORES):
        t0 = r * TC
        buf = np.zeros((XCW, D), np.float32)
        # window cols w=0..258 <-> x[t0-4+w]; col 259 stays zero
        lo = t0 - W
        src_lo = max(0, lo)
        buf[src_lo - lo:NW - 1] = xb[src_lo:t0 + TC - 1]
        # complement: every t outside [t0-1, t0+254]
        comp = np.concatenate([np.arange(0, max(0, t0 - 1)),
                               np.arange(t0 + TC - 1, T)])
        buf[NW:NW + len(comp)] = xb[comp]
        consts = cbase.copy()
        if r == 0:
            consts[0:V, C_E:C_E + W - 1] = E_NEG
        in_maps.append({"xall": _to_dev_layout(buf), "consts": consts,
                        "wois16": wois16})
    return in_maps


def unshard_output(results):
    y = np.empty((T, V), np.float32)
    for r in range(N_CORES):
        y[r * TC:(r + 1) * TC, :] = results[r]["yg"].T
    return y[None]


def run(inputs, trace=False, **kw):
    nc = _get_nc()
    in_maps = make_in_maps(inputs["x"], inputs["Uo"], inputs["Co"],
                           inputs["Wo"])
    res = bass_utils.run_bass_kernel_spmd(
        nc, in_maps, core_ids=list(range(N_CORES)), trace=trace, **kw)
    return unshard_output(res.results), res


def kernel(**inputs):
    out, _ = run(inputs)
    return out
